# revision 56
# baseline (speedup 1.0000x reference)
"""Multi-head self-attention (B=4, S=2048, D=1024, H=16) on 8 NeuronCores.

Sharding: data-parallel over batch (4 groups) x tensor-parallel over heads
(2 groups of 8 heads).  Core c handles batch b=c//2, head-group g=c%2.
Each core computes its 8 heads' attention plus a partial out-projection;
the host sums the two partials per batch, transposes, adds out_b.

Per-core schedule (v4 — engine-balance rewrite, 498us -> 361us in the
TimelineSim cost model):
  - fp32r on the PE for x/q/k/v (12-bit-mantissa fp32, single-pass full
    rate); otn/wo/outp in bf16 (out-projection rel-err ~4e-3 << 2e-2)
  - phase A: xT streamed in 512-col chunks, one DMA per chunk (xT is
    host-packed [128, ND, S] so a chunk is a single descriptor run);
    v projection and pair-0 q/k interleaved per chunk, PE starts ~6.5us
  - attention runs CH=512 chunks: both heads' scores go to ONE psum
    tile -> ONE [128,1024] exp per key-tile iteration, so the PE's
    critical path crosses a single ACT semaphore per iteration;
    the AV is software-pipelined TWO iterations behind (pt ring bufs=3)
    so its exp semaphore is always already satisfied; chunk-tail AVs
    carry into the next chunk's prologue
  - q/k projections are chopped into single-matmul micro steps (own
    2-slot PSUM ring) emitted one per iteration between the scores and
    the AV; only the k slices + q j0 are projected eagerly (before the
    pair starts) — q j1..j3 run lazily inside the pair's own chunks
    0..2, which lets the ACT-bound last pair absorb its own projection
    and shortens the PE-bound phase A (wstream bufs=4 avoids a
    DMA-queue/PE deadlock cycle through the lazy readers)
  - softmax denominators ride the AV as the ones column (row 64);
    normalization per chunk: PSUM-escape copies on DVE, denominator
    broadcast via a DRAM bounce (partition-stride-0 reads), O_B's
    partition move via one SBUF-SBUF DMA, then an in-place
    reciprocal_approx_fast + multiply that are DEFERRED to the middle
    of the next chunk so their DMA wait never head-of-line-blocks the
    DVE queue
  - otn (normalized attention output) stays resident in SBUF: the out
    projection reads it directly (no DRAM round trip); out-projection
    batches for tokens 0:1024 run as fillers inside pair 3's last two
    chunks, the rest alternate ACT/DVE copies in the final phase
  - qkT is ping-ponged (2 pairs) instead of holding all 4 pairs
Weights/outputs use host-prepacked tiled layouts so every DMA is
contiguous; walrus requires Bacc.compile() for the 1-wait-per-
instruction sync legalization.
"""

import numpy as np

_B, _S, _D, _H = 4, 2048, 1024, 16
_FH = 512  # local feature dims per core (8 heads x 64)
_ND = _D // 128
_NPAIR = _FH // 128
_NCORES = 8

_CACHE = {}


def _build(S):
    import concourse.bass as bass
    import concourse.bacc as bacc
    import concourse.tile as tile
    import concourse.mybir as mybir
    from contextlib import ExitStack

    f32 = mybir.dt.float32
    f32r = mybir.dt.float32r
    bf16 = mybir.dt.bfloat16
    Exp = mybir.ActivationFunctionType.Exp
    D, FH = _D, _FH
    ND = D // 128            # contraction tiles for the projections
    NPAIR = FH // 128        # head pairs
    NKT = S // 128           # key tiles
    CH = min(512, S)         # tq chunk
    NCH = S // CH
    HW = min(512, CH)        # matmul moving free dim
    NHALF = CH // HW
    TS = min(512, S)         # projection t-slice
    NTS = S // TS
    NH = FH // 64            # local heads
    FHA = NH * 65            # v width incl. per-head ones column
    XC = min(512, S)
    NXC = S // XC

    nc = bacc.Bacc("TRN2", target_bir_lowering=False, debug=False)

    xT_d = nc.dram_tensor("xT", [128, ND, S], f32r, kind="ExternalInput")
    wq_d = nc.dram_tensor("wq", [NPAIR, 128, ND, 128], f32r, kind="ExternalInput")
    wk_d = nc.dram_tensor("wk", [NPAIR, 128, ND, 128], f32r, kind="ExternalInput")
    wv_d = nc.dram_tensor("wv", [128, ND, FHA], f32r, kind="ExternalInput")
    wo_d = nc.dram_tensor("wo", [ND, 128, NPAIR, 128], bf16, kind="ExternalInput")
    bq_d = nc.dram_tensor("bq", [128, NPAIR], f32, kind="ExternalInput")
    bk_d = nc.dram_tensor("bk", [128, NPAIR], f32, kind="ExternalInput")
    bv_d = nc.dram_tensor("bv", [1, FHA], f32r, kind="ExternalInput")
    onr_d = nc.dram_tensor("onesr", [1, 128], f32r, kind="ExternalInput")
    outp_d = nc.dram_tensor("outp", [ND, 128, S], bf16, kind="ExternalOutput")
    v_d = nc.dram_tensor("v_scr", [128, NKT, FHA - 130], f32r)

    with tile.TileContext(nc) as tc, ExitStack() as top:
        consts = top.enter_context(tc.tile_pool(name="consts", bufs=1))
        ps = top.enter_context(tc.tile_pool(name="ps", bufs=2, space="PSUM"))

        ones_row = consts.tile([1, 128], f32r)
        bv_sb = consts.tile([1, FHA], f32r)
        bqk_sb = consts.tile([128, 2 * NPAIR], f32)
        # dummy exp so the ACT table set loads during the ramp, not at the
        # first real softmax exp inside the attention window
        warm = consts.tile([1, 8], f32)
        nc.vector.memset(warm, 0.0)
        nc.scalar.activation(out=warm, in_=warm, func=Exp)

        qkT_pool = top.enter_context(tc.tile_pool(name="qk", bufs=1))
        qkT = qkT_pool.tile([128, 2, 2, S], f32r)          # [f%128, p%2, q/k, t]
        vstream = top.enter_context(tc.tile_pool(name="vstream", bufs=2))
        wstream = top.enter_context(tc.tile_pool(name="wstream", bufs=4))

        def qk_batch(p, j, which, w_sb):
            """One q-or-k projection batch: 8 accumulating matmuls + bias."""
            pps = ps.tile([128, TS], f32, tag="f")
            for d in range(ND):
                nc.tensor.matmul(
                    pps,
                    lhsT=w_sb[:, d, :],
                    rhs=xT_sb[:, d, j * TS:(j + 1) * TS],
                    start=(d == 0),
                    stop=(d == ND - 1),
                )
            nc.vector.tensor_scalar_add(
                out=qkT[:, p % 2, which, j * TS:(j + 1) * TS],
                in0=pps,
                scalar1=bqk_sb[:, which * NPAIR + p:which * NPAIR + p + 1],
            )

        def load_pair(p):
            wq_sb = wstream.tile([128, ND, 128], f32r, tag="w")
            nc.sync.dma_start(out=wq_sb, in_=wq_d[p])
            wk_sb = wstream.tile([128, ND, 128], f32r, tag="w")
            nc.sync.dma_start(out=wk_sb, in_=wk_d[p])
            if p == 0:
                v_p = v_p0
            else:
                v_p = vstream.tile([128, NKT, 130], f32r, tag="vp")
                nc.sync.dma_start(
                    out=v_p, in_=v_d[:, :, (p - 1) * 130:p * 130])
            return (wq_sb, wk_sb), v_p

        with tc.tile_pool(name="xtp", bufs=1) as xtp:
            xT_sb = xtp.tile([128, ND, S], f32r)
            v_p0 = vstream.tile([128, NKT, 130], f32r, tag="vp")

            with tc.tile_pool(name="wvp", bufs=1) as wvp, \
                    tc.tile_pool(name="vst", bufs=16) as vst:
                wv_sb = wvp.tile([128, ND, FHA], f32r)

                # ----- startup DMA priority order: xT strip 0, then wv per-d
                # (v t0's d-matmuls chase the wv arrivals), remaining strips,
                # pair-0 weights, deferred consts; later xT chunks are
                # emitted inside the phase-A loop -----
                nc.sync.dma_start(
                    out=xT_sb[:, :, 0:128], in_=xT_d[:, :, 0:128])
                for d in range(ND):
                    nc.sync.dma_start(out=wv_sb[:, d, :], in_=wv_d[:, d, :])
                    if d == 3:      # consts needed by the first bias matmul
                        nc.sync.dma_start(out=ones_row, in_=onr_d[:])
                        nc.sync.dma_start(out=bv_sb, in_=bv_d[:])
                for s in range(1, XC // 128):
                    nc.sync.dma_start(
                        out=xT_sb[:, :, s * 128:(s + 1) * 128],
                        in_=xT_d[:, :, s * 128:(s + 1) * 128])
                w0 = load_pair(0)[0]
                if NXC > 1:                       # chunk 1 right after the
                    # weights, in halves so v t4/t5 can start sooner
                    nc.sync.dma_start(
                        out=xT_sb[:, :, XC:XC + XC // 2],
                        in_=xT_d[:, :, XC:XC + XC // 2])
                    nc.sync.dma_start(
                        out=xT_sb[:, :, XC + XC // 2:2 * XC],
                        in_=xT_d[:, :, XC + XC // 2:2 * XC])
                nc.sync.dma_start(out=bqk_sb[:, 0:NPAIR], in_=bq_d[:])
                nc.sync.dma_start(out=bqk_sb[:, NPAIR:2 * NPAIR], in_=bk_d[:])

                # ----- phase A: v projection + pair-0 q/k, interleaved -----
                vsplits = [(0, min(512, FHA))]
                if FHA > 512:
                    vsplits.append((512, FHA - 512))
                for c in range(NXC):
                    if c + 2 < NXC:                   # chunk c+2 in flight
                        nc.sync.dma_start(
                            out=xT_sb[:, :, (c + 2) * XC:(c + 3) * XC],
                            in_=xT_d[:, :, (c + 2) * XC:(c + 3) * XC])
                    for t in range(4 * c, 4 * c + 4):
                        vps = ps.tile([128, FHA], f32, tag="s")
                        for c0, cw in vsplits:
                            for d in range(ND):
                                nc.tensor.matmul(
                                    vps[:, c0:c0 + cw],
                                    lhsT=xT_sb[:, d, t * 128:(t + 1) * 128],
                                    rhs=wv_sb[:, d, c0:c0 + cw],
                                    start=(d == 0),
                                    stop=False,
                                )
                            nc.tensor.matmul(
                                vps[:, c0:c0 + cw], lhsT=ones_row,
                                rhs=bv_sb[:, c0:c0 + cw], start=False, stop=True,
                            )
                        nc.vector.tensor_copy(
                            out=v_p0[:, t, :], in_=vps[:, 0:130])
                        v_st = vst.tile([128, FHA - 130], f32r, tag="vs")
                        nc.scalar.copy(out=v_st, in_=vps[:, 130:FHA])
                        nc.sync.dma_start(out=v_d[:, t, :], in_=v_st)
                    qk_batch(0, c, 1, w0[1])   # k slice c
                    if c == 0:
                        qk_batch(0, c, 0, w0[0])   # q slice 0 (j1..j3 lazy)

            # ----- attention: pairs 0..3, software-pipelined -----
            ph2 = ExitStack()
            otn_pool = ph2.enter_context(tc.tile_pool(name="otn", bufs=1))
            otn = otn_pool.tile([128, NPAIR, S], bf16)     # resident attn output
            pt_pool = ph2.enter_context(tc.tile_pool(name="pt", bufs=3))
            nrm_pool = ph2.enter_context(tc.tile_pool(name="nrm", bufs=4))
            wo_pool = ph2.enter_context(tc.tile_pool(name="wop", bufs=8))
            st_pool = ph2.enter_context(tc.tile_pool(name="st", bufs=3))
            rs_pool = ph2.enter_context(tc.tile_pool(name="rsp", bufs=2))
            stv_pool = ph2.enter_context(tc.tile_pool(name="stv", bufs=2))
            dr_pool = ph2.enter_context(
                tc.tile_pool(name="dr", bufs=2, space="DRAM"))
            def make_qk_fillers(p, w_tiles):
                """Micro-step emitters for pair p's q/k projection (k first).
                One N=512 d-matmul per step; 64 steps per pair = one per
                i-iteration.  The accumulator lives in its own 2-slot PSUM
                ring so spreading steps across iterations cannot jam the
                score-tile ring."""
                steps = []
                cell = {}

                def step(j, which, w_sb, d):
                    if d == 0:
                        cell["pps"] = ps.tile(
                            [128, TS], f32, tag="f", name="fpps")
                    nc.tensor.matmul(
                        cell["pps"],
                        lhsT=w_sb[:, d, :],
                        rhs=xT_sb[:, d, j * TS:(j + 1) * TS],
                        start=(d == 0),
                        stop=(d == ND - 1),
                    )
                    if d == ND - 1:
                        nc.vector.tensor_scalar_add(
                            out=qkT[:, p % 2, which, j * TS:(j + 1) * TS],
                            in0=cell["pps"],
                            scalar1=bqk_sb[:, which * NPAIR + p:
                                           which * NPAIR + p + 1],
                        )

                def unit(j, which):
                    w_sb = w_tiles[0] if which == 0 else w_tiles[1]
                    return [lambda j=j, w=which, ws=w_sb, d=d:
                            step(j, w, ws, d) for d in range(ND)]

                # eager part (must finish before pair p starts): all k
                # slices + q j0.  The q j1..j3 slices are only read by
                # pair p's chunks 1..3 and are hosted lazily inside pair
                # p's own chunks 0..2 (returned separately).
                eager = []
                for j in range(NTS):
                    eager += unit(j, 1)
                eager += unit(0, 0)
                lazy = []
                for j in range(1, NTS):
                    lazy.append(unit(j, 0))
                return eager, lazy

            wo_tiles = {}

            def load_wo(et):
                wo_sb = wo_pool.tile([128, NPAIR, 128], bf16, tag="wo")
                nc.sync.dma_start(out=wo_sb, in_=wo_d[et])
                wo_tiles[et] = wo_sb

            OW = 1024          # out-projection batch token width

            def outproj_batch(et, jj, copy_eng, split=False):
                ops = ps.tile([128, OW], f32, tag="s")
                for h in range(2):
                    j = 2 * jj + h
                    for p in range(NPAIR):
                        nc.tensor.matmul(
                            ops[:, h * 512:(h + 1) * 512],
                            lhsT=wo_tiles[et][:, p, :],
                            rhs=otn[:, p, j * 512:(j + 1) * 512],
                            start=(p == 0),
                            stop=(p == NPAIR - 1),
                        )
                if split:
                    if copy_eng == "v":
                        st = stv_pool.tile([128, OW], bf16, tag="sv")
                    else:
                        st = st_pool.tile([128, OW], bf16, tag="st")
                    nc.scalar.copy(out=st[:, 0:512], in_=ops[:, 0:512])
                    nc.vector.tensor_copy(
                        out=st[:, 512:1024], in_=ops[:, 512:1024])
                    nc.sync.dma_start(
                        out=outp_d[et][:, 2 * jj * 512:(2 * jj + 1) * 512],
                        in_=st[:, 0:512])
                    nc.sync.dma_start(
                        out=outp_d[et][:, (2 * jj + 1) * 512:(2 * jj + 2) * 512],
                        in_=st[:, 512:1024])
                    return
                if copy_eng == "v":
                    st = stv_pool.tile([128, OW], bf16, tag="sv")
                    nc.vector.tensor_copy(out=st, in_=ops)
                else:
                    st = st_pool.tile([128, OW], bf16, tag="st")
                    nc.scalar.copy(out=st, in_=ops)
                nc.sync.dma_start(
                    out=outp_d[et][:, 2 * jj * 512:(2 * jj + 2) * 512], in_=st)

            def attention_chunk(p, ch, v_p, carry, fillers,
                                fill_at=frozenset(range(NKT))):
                """Emit one CH-token chunk; returns the carry closure that the
                next chunk's prologue invokes (tail AV + normalization)."""
                t0 = ch * CH
                oA = ps.tile([128, CH], f32, tag="o")
                oB = ps.tile([128, CH], f32, tag="o")
                slot = p % 2
                prev = None

                def emit_scores(i):
                    # both heads' scores into one PSUM tile -> ONE exp per
                    # iteration (single semaphore on the PE's critical path)
                    s2 = ps.tile([128, 2 * CH], f32, tag="s")
                    kslc = slice(i * 128, (i + 1) * 128)
                    for half, lo in ((0, 0), (1, 64)):
                        nc.tensor.matmul(
                            s2[:, half * CH:(half + 1) * CH],
                            lhsT=qkT[lo:lo + 64, slot, 1, kslc],
                            rhs=qkT[lo:lo + 64, slot, 0, t0:t0 + CH],
                            start=True, stop=True,
                            tile_position=(lo, 0),
                        )
                    pt = pt_pool.tile([128, 2 * CH], f32r, tag="pt")
                    nc.scalar.activation(out=pt, in_=s2, func=Exp, scale=0.125)
                    return pt

                def emit_av(rec, half):
                    i, pt = rec
                    first, last = (i == 0), (i == NKT - 1)
                    ox = oA if half == 0 else oB
                    vw = slice(0, 65) if half == 0 else slice(65, 130)
                    nc.tensor.matmul(
                        ox[0:65, :], lhsT=v_p[:, i, vw],
                        rhs=pt[:, half * CH:(half + 1) * CH],
                        start=first, stop=last,
                    )

                pend = []          # AV emission lags TWO iterations so the
                for i in range(NKT):   # pt sem is always satisfied already
                    pt = emit_scores(i)
                    if i == 0 and carry is not None:
                        carry(0)
                    if i in fill_at and fillers:
                        fillers.pop(0)()
                    if len(pend) >= 2:
                        emit_av(pend[0], 0)
                    if i == 0 and carry is not None:
                        carry(1)
                    if len(pend) >= 2:
                        emit_av(pend.pop(0), 1)
                    if i == 8 and carry is not None:
                        carry(2)
                        carry = None
                    pend.append((i, pt))

                nrm = {}

                def new_carry(phase):
                    if phase == 0:
                        emit_av(pend[0], 0)
                        emit_av(pend[0], 1)
                        return
                    if phase == 1:
                        emit_av(pend[1], 0)
                        emit_av(pend[1], 1)
                        # --- normalization part 1: PSUM escape + denominator
                        # broadcast via a DRAM bounce (DMA-only tail) ---
                        aS = nrm_pool.tile([128, CH], f32, tag="n")
                        nc.vector.tensor_copy(out=aS[0:65, :], in_=oA[0:65, :])
                        bS = nrm_pool.tile([128, CH], f32, tag="n")
                        nc.vector.tensor_copy(out=bS[0:65, :], in_=oB[0:65, :])
                        dscr = dr_pool.tile([2, CH], f32, tag="d")
                        nc.sync.dma_start(out=dscr[0:1, :], in_=aS[64:65, :])
                        nc.sync.dma_start(out=dscr[1:2, :], in_=bS[64:65, :])
                        nc.sync.dma_start(out=aS[64:128, :], in_=bS[0:64, :])
                        rS = rs_pool.tile([128, CH], f32, tag="rs")
                        nc.sync.dma_start(
                            out=rS[0:64, :],
                            in_=dscr[0:1, :].to_broadcast([64, CH]))
                        nc.sync.dma_start(
                            out=rS[64:128, :],
                            in_=dscr[1:2, :].to_broadcast([64, CH]))
                        nrm.update(aS=aS, rS=rS)
                        return
                    # phase 2 (deferred to mid-next-chunk so the recip's DMA
                    # wait never head-of-line-blocks the DVE queue)
                    nc.vector.reciprocal_approx_fast(
                        out=nrm["rS"], in_=nrm["rS"])
                    nc.vector.tensor_mul(
                        out=otn[:, p, t0:t0 + CH], in0=nrm["aS"], in1=nrm["rS"])
                return new_carry

            w_cur, v_cur = w0, v_p0
            lazy0 = make_qk_fillers(0, w0)[1]
            lazy_cur = lazy0            # pair p's own q j1..j3 slices
            carry = None
            for p in range(NPAIR):
                if p + 1 < NPAIR:
                    w_nxt, v_nxt = load_pair(p + 1)
                    eager, lazy_nxt = make_qk_fillers(p + 1, w_nxt)
                else:
                    w_nxt = v_nxt = None
                    for et in range(8):
                        load_wo(et)
                    eager, lazy_nxt = [], None
                for ch in range(NCH):
                    if p == NPAIR - 1 and ch >= NCH - 2:
                        # out-proj fillers over tokens 0:1024 (normed by now);
                        # placed late so the pending chunk norm has completed
                        et0 = 2 * (ch - (NCH - 2))
                        fl = [lambda et=et0: outproj_batch(et, 0, "v"),
                              lambda et=et0 + 1: outproj_batch(et, 0, "v")]
                        carry = attention_chunk(
                            p, ch, v_cur, carry, fl,
                            fill_at=frozenset({10, 13}))
                        continue
                    # lazy q j(ch+1) first (read by the NEXT chunk), then
                    # this chunk's share of the next pair's eager steps
                    fl = []
                    if lazy_cur:
                        if p == NPAIR - 1:
                            parts = {0: [0], 1: [1, 2]}.get(ch, [])
                        else:
                            parts = [ch] if ch < len(lazy_cur) else []
                        for ix in parts:
                            fl += lazy_cur[ix]
                    take = NKT - len(fl)
                    fl += eager[:take]
                    eager = eager[take:]
                    if len(fl) < NKT:    # spread sparse fillers evenly
                        fa = frozenset(
                            round(k * NKT / len(fl)) for k in range(len(fl)))
                        carry = attention_chunk(
                            p, ch, v_cur, carry, fl, fill_at=fa)
                    else:
                        carry = attention_chunk(p, ch, v_cur, carry, fl)
                w_cur, v_cur = w_nxt, v_nxt
                lazy_cur = lazy_nxt
            carry(0)
            carry(1)
            carry(2)

            # ----- out projection (remaining batches; copies alternate
            # ACT / DVE so the tail drains through two engines) -----
            alt = 0
            for et in range(2, 8):
                outproj_batch(et, 0, "s" if alt % 2 == 0 else "v")
                alt += 1
            for et in range(8):
                outproj_batch(et, 1, "s" if alt % 2 == 0 else "v",
                              split=(et >= 6))
                alt += 1
            ph2.close()

    nc.compile()
    return nc


def _get_nc(S=_S):
    if S not in _CACHE:
        _CACHE[S] = _build(S)
    return _CACHE[S]


def _c32(a):
    return np.ascontiguousarray(a, dtype=np.float32)


def _bf16(a):
    import ml_dtypes
    return np.ascontiguousarray(
        np.asarray(a, dtype=np.float32).astype(ml_dtypes.bfloat16))


def _round_f32r(a):
    """Round fp32 -> nearest fp32r (12-bit mantissa) so PE fp32r matmuls
    see properly rounded operands."""
    a = _c32(a)
    try:
        from neuron_dtypes._impl.fp32r import cast_fp32_to_fp32r
        flat = a.reshape(-1).view(np.uint32)
        out = np.asarray(cast_fp32_to_fp32r(flat.size, flat), dtype=np.uint32)
        return np.ascontiguousarray(out.view(np.float32).reshape(a.shape))
    except Exception:
        return a


def make_in_map(xT, wqT, wkT, wvT, woT, bq, bk, bv):
    """Pack one core's inputs into the kernel's tiled DRAM layouts."""
    D, FH, ND, NPAIR = _D, _FH, _ND, _NPAIR
    NH = FH // 64
    FHA = NH * 65
    wva = np.zeros((D, FHA), dtype=np.float32)
    bva = np.zeros((1, FHA), dtype=np.float32)
    for h in range(NH):
        wva[:, h * 65:h * 65 + 64] = np.asarray(wvT)[:, h * 64:(h + 1) * 64]
        bva[0, h * 65:h * 65 + 64] = np.asarray(bv)[h * 64:(h + 1) * 64]
        bva[0, h * 65 + 64] = 1.0
    return {
        "xT": _round_f32r(np.asarray(xT).reshape(ND, 128, -1).transpose(1, 0, 2)),
        "wq": _round_f32r(np.asarray(wqT).reshape(ND, 128, NPAIR, 128).transpose(2, 1, 0, 3)),
        "wk": _round_f32r(np.asarray(wkT).reshape(ND, 128, NPAIR, 128).transpose(2, 1, 0, 3)),
        "wv": _round_f32r(wva.reshape(ND, 128, FHA).transpose(1, 0, 2)),
        "wo": _bf16(np.asarray(woT).reshape(NPAIR, 128, ND, 128).transpose(2, 1, 0, 3)),
        "bq": _c32(np.asarray(bq).reshape(_NPAIR, 128).T),
        "bk": _c32(np.asarray(bk).reshape(_NPAIR, 128).T),
        "bv": _round_f32r(bva),
        "onesr": np.ones((1, 128), dtype=np.float32),
    }


def unpack_out(outp_tiled, S=_S):
    """[ND, 128, S] tiled partial -> [D, S]."""
    return np.asarray(outp_tiled, dtype=np.float32).reshape(_D, S)


def _shard_inputs(x, in_proj_weight, in_proj_bias, out_w):
    w = np.asarray(in_proj_weight)
    b = np.asarray(in_proj_bias)
    ow = np.asarray(out_w)
    in_maps = []
    for c in range(_NCORES):
        bi, g = divmod(c, 2)
        sl = slice(g * _FH, (g + 1) * _FH)
        in_maps.append(make_in_map(
            xT=np.asarray(x[bi]).T,
            wqT=w[0 * _D:1 * _D][sl].T,
            wkT=w[1 * _D:2 * _D][sl].T,
            wvT=w[2 * _D:3 * _D][sl].T,
            woT=ow[:, sl].T,
            bq=b[0 * _D:1 * _D][sl],
            bk=b[1 * _D:2 * _D][sl],
            bv=b[2 * _D:3 * _D][sl],
        ))
    return in_maps


LAST_RESULTS = None


def kernel(x, in_proj_weight, in_proj_bias, out_w, out_b):
    global LAST_RESULTS
    from concourse.bass_utils import run_bass_kernel_spmd
    import os

    nc = _get_nc()
    in_maps = _shard_inputs(x, in_proj_weight, in_proj_bias, out_w)
    trace = os.environ.get("BASS_TRACE", "0") not in ("", "0")
    res = run_bass_kernel_spmd(
        nc, in_maps, core_ids=list(range(_NCORES)), trace=trace
    )
    LAST_RESULTS = res
    out_b = np.asarray(out_b, dtype=np.float32)
    out = np.empty((_B, _S, _D), dtype=np.float32)
    for b in range(_B):
        part = (unpack_out(res.results[2 * b]["outp"])
                + unpack_out(res.results[2 * b + 1]["outp"]))
        out[b] = part.T + out_b
    return out


# revision 59
# speedup vs baseline: 1.0236x; 1.0236x over previous
"""Multi-head self-attention (B=4, S=2048, D=1024, H=16) on 8 NeuronCores.

Sharding: data-parallel over batch (4 groups) x tensor-parallel over heads
(2 groups of 8 heads).  Core c handles batch b=c//2, head-group g=c%2.
Each core computes its 8 heads' attention plus a partial out-projection;
the host sums the two partials per batch, transposes, adds out_b.

Per-core schedule (v4 — engine-balance rewrite, 498us -> 361us in the
TimelineSim cost model):
  - fp32r on the PE for x/q/k/v (12-bit-mantissa fp32, single-pass full
    rate); otn/wo/outp in bf16 (out-projection rel-err ~4e-3 << 2e-2)
  - phase A: xT streamed in 512-col chunks, one DMA per chunk (xT is
    host-packed [128, ND, S] so a chunk is a single descriptor run);
    v projection and pair-0 q/k interleaved per chunk, PE starts ~6.5us
  - attention runs CH=512 chunks: both heads' scores go to ONE psum
    tile -> ONE [128,1024] exp per key-tile iteration, so the PE's
    critical path crosses a single ACT semaphore per iteration;
    the AV is software-pipelined TWO iterations behind (pt ring bufs=3)
    so its exp semaphore is always already satisfied; chunk-tail AVs
    carry into the next chunk's prologue
  - q/k projections are chopped into single-matmul micro steps (own
    2-slot PSUM ring) emitted one per iteration between the scores and
    the AV; only the k slices + q j0 are projected eagerly (before the
    pair starts) — q j1..j3 run lazily inside the pair's own chunks
    0..2, which lets the ACT-bound last pair absorb its own projection
    and shortens the PE-bound phase A (wstream bufs=4 avoids a
    DMA-queue/PE deadlock cycle through the lazy readers)
  - softmax denominators ride the AV as the ones column (row 64);
    normalization per chunk: PSUM-escape copies on DVE, denominator
    broadcast via a DRAM bounce (partition-stride-0 reads), O_B's
    partition move via one SBUF-SBUF DMA, then an in-place
    reciprocal_approx_fast + multiply that are DEFERRED to the middle
    of the next chunk so their DMA wait never head-of-line-blocks the
    DVE queue
  - otn (normalized attention output) stays resident in SBUF: the out
    projection reads it directly (no DRAM round trip); out-projection
    batches for tokens 0:1024 run as fillers inside pair 3's last two
    chunks, the rest alternate ACT/DVE copies in the final phase
  - qkT is ping-ponged (2 pairs) instead of holding all 4 pairs
Weights/outputs use host-prepacked tiled layouts so every DMA is
contiguous; walrus requires Bacc.compile() for the 1-wait-per-
instruction sync legalization.
"""

import numpy as np

_B, _S, _D, _H = 4, 2048, 1024, 16
_FH = 512  # local feature dims per core (8 heads x 64)
_ND = _D // 128
_NPAIR = _FH // 128
_NCORES = 8

_CACHE = {}


def _build(S):
    import concourse.bass as bass
    import concourse.bacc as bacc
    import concourse.tile as tile
    import concourse.mybir as mybir
    from contextlib import ExitStack

    f32 = mybir.dt.float32
    f32r = mybir.dt.float32r
    bf16 = mybir.dt.bfloat16
    Exp = mybir.ActivationFunctionType.Exp
    D, FH = _D, _FH
    ND = D // 128            # contraction tiles for the projections
    NPAIR = FH // 128        # head pairs
    NKT = S // 128           # key tiles
    CH = min(512, S)         # tq chunk
    NCH = S // CH
    HW = min(512, CH)        # matmul moving free dim
    NHALF = CH // HW
    TS = min(512, S)         # projection t-slice
    NTS = S // TS
    NH = FH // 64            # local heads
    FHA = NH * 65            # v width incl. per-head ones column
    XC = min(512, S)
    NXC = S // XC

    nc = bacc.Bacc("TRN2", target_bir_lowering=False, debug=False)

    xT_d = nc.dram_tensor("xT", [128, ND, S], f32r, kind="ExternalInput")
    wq_d = nc.dram_tensor("wq", [NPAIR, 128, ND, 128], f32r, kind="ExternalInput")
    wk_d = nc.dram_tensor("wk", [NPAIR, 128, ND, 128], f32r, kind="ExternalInput")
    wv_d = nc.dram_tensor("wv", [128, ND, FHA], f32r, kind="ExternalInput")
    wo_d = nc.dram_tensor("wo", [ND, 128, NPAIR, 128], bf16, kind="ExternalInput")
    bq_d = nc.dram_tensor("bq", [128, NPAIR], f32, kind="ExternalInput")
    bk_d = nc.dram_tensor("bk", [128, NPAIR], f32, kind="ExternalInput")
    bv_d = nc.dram_tensor("bv", [1, FHA], f32r, kind="ExternalInput")
    onr_d = nc.dram_tensor("onesr", [1, 128], f32r, kind="ExternalInput")
    outp_d = nc.dram_tensor("outp", [ND, 128, S], bf16, kind="ExternalOutput")
    v_d = nc.dram_tensor("v_scr", [128, NKT, FHA - 130], f32r)

    with tile.TileContext(nc) as tc, ExitStack() as top:
        consts = top.enter_context(tc.tile_pool(name="consts", bufs=1))
        ps = top.enter_context(tc.tile_pool(name="ps", bufs=2, space="PSUM"))

        ones_row = consts.tile([1, 128], f32r)
        bv_sb = consts.tile([1, FHA], f32r)
        bqk_sb = consts.tile([128, 2 * NPAIR], f32)
        # dummy exp so the ACT table set loads during the ramp, not at the
        # first real softmax exp inside the attention window
        warm = consts.tile([1, 8], f32)
        nc.vector.memset(warm, 0.0)
        nc.scalar.activation(out=warm, in_=warm, func=Exp)

        qkT_pool = top.enter_context(tc.tile_pool(name="qk", bufs=1))
        qkT = qkT_pool.tile([128, 2, 2, S], f32r)          # [f%128, p%2, q/k, t]
        vstream = top.enter_context(tc.tile_pool(name="vstream", bufs=2))
        wstream = top.enter_context(tc.tile_pool(name="wstream", bufs=4))

        def qk_batch(p, j, which, w_sb):
            """One q-or-k projection batch: 8 accumulating matmuls + bias."""
            pps = ps.tile([128, TS], f32, tag="f")
            for d in range(ND):
                nc.tensor.matmul(
                    pps,
                    lhsT=w_sb[:, d, :],
                    rhs=xT_sb[:, d, j * TS:(j + 1) * TS],
                    start=(d == 0),
                    stop=(d == ND - 1),
                )
            nc.vector.tensor_scalar_add(
                out=qkT[:, p % 2, which, j * TS:(j + 1) * TS],
                in0=pps,
                scalar1=bqk_sb[:, which * NPAIR + p:which * NPAIR + p + 1],
            )

        def load_pair(p):
            wq_sb = wstream.tile([128, ND, 128], f32r, tag="w")
            nc.sync.dma_start(out=wq_sb, in_=wq_d[p])
            wk_sb = wstream.tile([128, ND, 128], f32r, tag="w")
            nc.sync.dma_start(out=wk_sb, in_=wk_d[p])
            if p == 0:
                v_p = v_p0
            else:
                v_p = vstream.tile([128, NKT, 130], f32r, tag="vp")
                nc.sync.dma_start(
                    out=v_p, in_=v_d[:, :, (p - 1) * 130:p * 130])
            return (wq_sb, wk_sb), v_p

        with tc.tile_pool(name="xtp", bufs=1) as xtp:
            xT_sb = xtp.tile([128, ND, S], f32r)
            v_p0 = vstream.tile([128, NKT, 130], f32r, tag="vp")

            with tc.tile_pool(name="wvp", bufs=1) as wvp, \
                    tc.tile_pool(name="vst", bufs=16) as vst:
                wv_sb = wvp.tile([128, ND, FHA], f32r)

                # ----- startup DMA priority order: xT strip 0, then wv per-d
                # (v t0's d-matmuls chase the wv arrivals), remaining strips,
                # pair-0 weights, deferred consts; later xT chunks are
                # emitted inside the phase-A loop -----
                nc.sync.dma_start(
                    out=xT_sb[:, :, 0:128], in_=xT_d[:, :, 0:128])
                for d in range(ND):
                    nc.sync.dma_start(out=wv_sb[:, d, :], in_=wv_d[:, d, :])
                    if d == 3:      # consts needed by the first bias matmul
                        nc.sync.dma_start(out=ones_row, in_=onr_d[:])
                        nc.sync.dma_start(out=bv_sb, in_=bv_d[:])
                for s in range(1, XC // 128):
                    nc.sync.dma_start(
                        out=xT_sb[:, :, s * 128:(s + 1) * 128],
                        in_=xT_d[:, :, s * 128:(s + 1) * 128])
                w0 = load_pair(0)[0]
                if NXC > 1:                       # chunk 1 right after the
                    # weights, in halves so v t4/t5 can start sooner
                    nc.sync.dma_start(
                        out=xT_sb[:, :, XC:XC + XC // 2],
                        in_=xT_d[:, :, XC:XC + XC // 2])
                    nc.sync.dma_start(
                        out=xT_sb[:, :, XC + XC // 2:2 * XC],
                        in_=xT_d[:, :, XC + XC // 2:2 * XC])
                nc.sync.dma_start(out=bqk_sb[:, 0:NPAIR], in_=bq_d[:])
                nc.sync.dma_start(out=bqk_sb[:, NPAIR:2 * NPAIR], in_=bk_d[:])

                # ----- phase A: v projection + pair-0 q/k, interleaved -----
                vsplits = [(0, min(512, FHA))]
                if FHA > 512:
                    vsplits.append((512, FHA - 512))
                for c in range(NXC):
                    if c + 2 < NXC:                   # chunk c+2 in flight
                        nc.sync.dma_start(
                            out=xT_sb[:, :, (c + 2) * XC:(c + 3) * XC],
                            in_=xT_d[:, :, (c + 2) * XC:(c + 3) * XC])
                    for t in range(4 * c, 4 * c + 4):
                        vps = ps.tile([128, FHA], f32, tag="s")
                        for c0, cw in vsplits:
                            for d in range(ND):
                                nc.tensor.matmul(
                                    vps[:, c0:c0 + cw],
                                    lhsT=xT_sb[:, d, t * 128:(t + 1) * 128],
                                    rhs=wv_sb[:, d, c0:c0 + cw],
                                    start=(d == 0),
                                    stop=False,
                                )
                            nc.tensor.matmul(
                                vps[:, c0:c0 + cw], lhsT=ones_row,
                                rhs=bv_sb[:, c0:c0 + cw], start=False, stop=True,
                            )
                        nc.vector.tensor_copy(
                            out=v_p0[:, t, :], in_=vps[:, 0:130])
                        v_st = vst.tile([128, FHA - 130], f32r, tag="vs")
                        nc.scalar.copy(out=v_st, in_=vps[:, 130:FHA])
                        nc.sync.dma_start(out=v_d[:, t, :], in_=v_st)
                    qk_batch(0, c, 1, w0[1])   # k slice c
                    if c == 0:
                        qk_batch(0, c, 0, w0[0])   # q slice 0 (j1..j3 lazy)

            # ----- attention: pairs 0..3, software-pipelined -----
            ph2 = ExitStack()
            otn_pool = ph2.enter_context(tc.tile_pool(name="otn", bufs=1))
            otn = otn_pool.tile([128, NPAIR, S], bf16)     # resident attn output
            pt_pool = ph2.enter_context(tc.tile_pool(name="pt", bufs=3))
            nrm_pool = ph2.enter_context(tc.tile_pool(name="nrm", bufs=4))
            wo_pool = ph2.enter_context(tc.tile_pool(name="wop", bufs=8))
            st_pool = ph2.enter_context(tc.tile_pool(name="st", bufs=3))
            rs_pool = ph2.enter_context(tc.tile_pool(name="rsp", bufs=2))
            stv_pool = ph2.enter_context(tc.tile_pool(name="stv", bufs=2))
            dr_pool = ph2.enter_context(
                tc.tile_pool(name="dr", bufs=2, space="DRAM"))
            def make_qk_fillers(p, w_tiles):
                """Micro-step emitters for pair p's q/k projection (k first).
                One N=512 d-matmul per step; 64 steps per pair = one per
                i-iteration.  The accumulator lives in its own 2-slot PSUM
                ring so spreading steps across iterations cannot jam the
                score-tile ring."""
                steps = []
                cell = {}

                def step(j, which, w_sb, d):
                    if d == 0:
                        cell["pps"] = ps.tile(
                            [128, TS], f32, tag="f", name="fpps")
                    nc.tensor.matmul(
                        cell["pps"],
                        lhsT=w_sb[:, d, :],
                        rhs=xT_sb[:, d, j * TS:(j + 1) * TS],
                        start=(d == 0),
                        stop=(d == ND - 1),
                    )
                    if d == ND - 1:
                        nc.vector.tensor_scalar_add(
                            out=qkT[:, p % 2, which, j * TS:(j + 1) * TS],
                            in0=cell["pps"],
                            scalar1=bqk_sb[:, which * NPAIR + p:
                                           which * NPAIR + p + 1],
                        )

                def unit(j, which):
                    w_sb = w_tiles[0] if which == 0 else w_tiles[1]
                    return [lambda j=j, w=which, ws=w_sb, d=d:
                            step(j, w, ws, d) for d in range(ND)]

                # eager part (must finish before pair p starts): all k
                # slices + q j0.  The q j1..j3 slices are only read by
                # pair p's chunks 1..3 and are hosted lazily inside pair
                # p's own chunks 0..2 (returned separately).
                eager = []
                for j in range(NTS):
                    eager += unit(j, 1)
                eager += unit(0, 0)
                lazy = []
                for j in range(1, NTS):
                    lazy.append(unit(j, 0))
                return eager, lazy

            wo_tiles = {}

            def make_outproj_steps(ets):
                """1-matmul out-projection micro steps for pair-3 slack.
                Each (et, j) unit: 4 accumulating N=512 matmuls in the free
                f-ring + a DVE copy and DMA.  Ordered all-j0-then-j1 so the
                j1 reads come after chunk 1's deferred norm multiply."""
                steps = []
                cell = {}

                def step(et, j, p):
                    if p == 0:
                        cell["ops"] = ps.tile(
                            [128, 512], f32, tag="f", name="ojp")
                    nc.tensor.matmul(
                        cell["ops"],
                        lhsT=wo_tiles[et][:, p, :],
                        rhs=otn[:, p, j * 512:(j + 1) * 512],
                        start=(p == 0),
                        stop=(p == NPAIR - 1),
                    )
                    if p == NPAIR - 1:
                        st = stv_pool.tile([128, 512], bf16, tag="sv",
                                           name="stj")
                        nc.vector.tensor_copy(out=st, in_=cell["ops"])
                        nc.sync.dma_start(
                            out=outp_d[et][:, j * 512:(j + 1) * 512], in_=st)

                for j in (0, 1):
                    for et in ets:
                        for p in range(NPAIR):
                            steps.append(
                                lambda et=et, j=j, p=p: step(et, j, p))
                return steps

            def load_wo(et):
                wo_sb = wo_pool.tile([128, NPAIR, 128], bf16, tag="wo")
                nc.sync.dma_start(out=wo_sb, in_=wo_d[et])
                wo_tiles[et] = wo_sb

            OW = 1024          # out-projection batch token width

            def outproj_batch(et, jj, copy_eng, split=False):
                ops = ps.tile([128, OW], f32, tag="s")
                for h in range(2):
                    j = 2 * jj + h
                    for p in range(NPAIR):
                        nc.tensor.matmul(
                            ops[:, h * 512:(h + 1) * 512],
                            lhsT=wo_tiles[et][:, p, :],
                            rhs=otn[:, p, j * 512:(j + 1) * 512],
                            start=(p == 0),
                            stop=(p == NPAIR - 1),
                        )
                if split:
                    if copy_eng == "v":
                        st = stv_pool.tile([128, OW], bf16, tag="sv")
                    else:
                        st = st_pool.tile([128, OW], bf16, tag="st")
                    nc.scalar.copy(out=st[:, 0:512], in_=ops[:, 0:512])
                    nc.vector.tensor_copy(
                        out=st[:, 512:1024], in_=ops[:, 512:1024])
                    nc.sync.dma_start(
                        out=outp_d[et][:, 2 * jj * 512:(2 * jj + 1) * 512],
                        in_=st[:, 0:512])
                    nc.sync.dma_start(
                        out=outp_d[et][:, (2 * jj + 1) * 512:(2 * jj + 2) * 512],
                        in_=st[:, 512:1024])
                    return
                if copy_eng == "v":
                    st = stv_pool.tile([128, OW], bf16, tag="sv")
                    nc.vector.tensor_copy(out=st, in_=ops)
                else:
                    st = st_pool.tile([128, OW], bf16, tag="st")
                    nc.scalar.copy(out=st, in_=ops)
                nc.sync.dma_start(
                    out=outp_d[et][:, 2 * jj * 512:(2 * jj + 2) * 512], in_=st)

            def attention_chunk(p, ch, v_p, carry, fillers,
                                fill_at=frozenset(range(NKT))):
                """Emit one CH-token chunk; returns the carry closure that the
                next chunk's prologue invokes (tail AV + normalization)."""
                t0 = ch * CH
                oA = ps.tile([128, CH], f32, tag="o")
                oB = ps.tile([128, CH], f32, tag="o")
                slot = p % 2
                prev = None

                def emit_scores(i):
                    # both heads' scores into one PSUM tile -> ONE exp per
                    # iteration (single semaphore on the PE's critical path)
                    s2 = ps.tile([128, 2 * CH], f32, tag="s")
                    kslc = slice(i * 128, (i + 1) * 128)
                    for half, lo in ((0, 0), (1, 64)):
                        nc.tensor.matmul(
                            s2[:, half * CH:(half + 1) * CH],
                            lhsT=qkT[lo:lo + 64, slot, 1, kslc],
                            rhs=qkT[lo:lo + 64, slot, 0, t0:t0 + CH],
                            start=True, stop=True,
                            tile_position=(lo, 0),
                        )
                    pt = pt_pool.tile([128, 2 * CH], f32r, tag="pt")
                    nc.scalar.activation(out=pt, in_=s2, func=Exp, scale=0.125)
                    return pt

                def emit_av(rec, half):
                    i, pt = rec
                    first, last = (i == 0), (i == NKT - 1)
                    ox = oA if half == 0 else oB
                    vw = slice(0, 65) if half == 0 else slice(65, 130)
                    nc.tensor.matmul(
                        ox[0:65, :], lhsT=v_p[:, i, vw],
                        rhs=pt[:, half * CH:(half + 1) * CH],
                        start=first, stop=last,
                    )

                pend = []          # AV emission lags TWO iterations so the
                for i in range(NKT):   # pt sem is always satisfied already
                    pt = emit_scores(i)
                    if i == 0 and carry is not None:
                        carry(0)
                    if i in fill_at and fillers:
                        fillers.pop(0)()
                    if len(pend) >= 2:
                        emit_av(pend[0], 0)
                    if i == 0 and carry is not None:
                        carry(1)
                    if len(pend) >= 2:
                        emit_av(pend.pop(0), 1)
                    if i == 8 and carry is not None:
                        carry(2)
                        carry = None
                    pend.append((i, pt))

                nrm = {}

                def new_carry(phase):
                    if phase == 0:
                        emit_av(pend[0], 0)
                        emit_av(pend[0], 1)
                        return
                    if phase == 1:
                        emit_av(pend[1], 0)
                        emit_av(pend[1], 1)
                        # --- normalization part 1: PSUM escape + denominator
                        # broadcast via a DRAM bounce (DMA-only tail) ---
                        aS = nrm_pool.tile([128, CH], f32, tag="n")
                        nc.vector.tensor_copy(out=aS[0:65, :], in_=oA[0:65, :])
                        bS = nrm_pool.tile([128, CH], f32, tag="n")
                        nc.vector.tensor_copy(out=bS[0:65, :], in_=oB[0:65, :])
                        dscr = dr_pool.tile([2, CH], f32, tag="d")
                        nc.sync.dma_start(out=dscr[0:1, :], in_=aS[64:65, :])
                        nc.sync.dma_start(out=dscr[1:2, :], in_=bS[64:65, :])
                        nc.sync.dma_start(out=aS[64:128, :], in_=bS[0:64, :])
                        rS = rs_pool.tile([128, CH], f32, tag="rs")
                        nc.sync.dma_start(
                            out=rS[0:64, :],
                            in_=dscr[0:1, :].to_broadcast([64, CH]))
                        nc.sync.dma_start(
                            out=rS[64:128, :],
                            in_=dscr[1:2, :].to_broadcast([64, CH]))
                        nrm.update(aS=aS, rS=rS)
                        return
                    # phase 2 (deferred to mid-next-chunk so the recip's DMA
                    # wait never head-of-line-blocks the DVE queue)
                    nc.vector.reciprocal_approx_fast(
                        out=nrm["rS"], in_=nrm["rS"])
                    nc.vector.tensor_mul(
                        out=otn[:, p, t0:t0 + CH], in0=nrm["aS"], in1=nrm["rS"])
                return new_carry

            w_cur, v_cur = w0, v_p0
            lazy0 = make_qk_fillers(0, w0)[1]
            lazy_cur = lazy0            # pair p's own q j1..j3 slices
            carry = None
            for p in range(NPAIR):
                if p + 1 < NPAIR:
                    w_nxt, v_nxt = load_pair(p + 1)
                    eager, lazy_nxt = make_qk_fillers(p + 1, w_nxt)
                else:
                    w_nxt = v_nxt = None
                    for et in range(8):
                        load_wo(et)
                    eager, lazy_nxt = [], None
                if p == NPAIR - 1:
                    osteps = make_outproj_steps(range(3))
                for ch in range(NCH):
                    if p == NPAIR - 1 and ch >= NCH - 2:
                        # out-proj micro steps over tokens 0:1024; first 16
                        # in chunk 2 (j1 reads land after the i==8 deferred
                        # norm), remaining 8 spread over chunk 3
                        fl = osteps[:16]
                        osteps = osteps[16:]
                        fa = (frozenset(range(NKT)) if len(fl) >= NKT else
                              frozenset(round(k * NKT / len(fl))
                                        for k in range(len(fl))))
                        carry = attention_chunk(
                            p, ch, v_cur, carry, fl, fill_at=fa)
                        continue
                    # lazy q j(ch+1) first (read by the NEXT chunk), then
                    # this chunk's share of the next pair's eager steps
                    fl = []
                    if lazy_cur:
                        if p == NPAIR - 1:
                            parts = {0: [0], 1: [1, 2]}.get(ch, [])
                        else:
                            parts = [ch] if ch < len(lazy_cur) else []
                        for ix in parts:
                            fl += lazy_cur[ix]
                    take = NKT - len(fl)
                    fl += eager[:take]
                    eager = eager[take:]
                    if len(fl) < NKT:    # spread sparse fillers evenly
                        fa = frozenset(
                            round(k * NKT / len(fl)) for k in range(len(fl)))
                        carry = attention_chunk(
                            p, ch, v_cur, carry, fl, fill_at=fa)
                    else:
                        carry = attention_chunk(p, ch, v_cur, carry, fl)
                w_cur, v_cur = w_nxt, v_nxt
                lazy_cur = lazy_nxt
            carry(0)
            carry(1)
            carry(2)

            # ----- out projection (remaining batches; copies alternate
            # ACT / DVE so the tail drains through two engines) -----
            alt = 0
            for et in range(3, 8):
                outproj_batch(et, 0, "s" if alt % 2 == 0 else "v")
                alt += 1
            for et in range(8):
                outproj_batch(et, 1, "s" if alt % 2 == 0 else "v",
                              split=(et >= 6))
                alt += 1
            ph2.close()

    nc.compile()
    return nc


def _get_nc(S=_S):
    if S not in _CACHE:
        _CACHE[S] = _build(S)
    return _CACHE[S]


def _c32(a):
    return np.ascontiguousarray(a, dtype=np.float32)


def _bf16(a):
    import ml_dtypes
    return np.ascontiguousarray(
        np.asarray(a, dtype=np.float32).astype(ml_dtypes.bfloat16))


def _round_f32r(a):
    """Round fp32 -> nearest fp32r (12-bit mantissa) so PE fp32r matmuls
    see properly rounded operands."""
    a = _c32(a)
    try:
        from neuron_dtypes._impl.fp32r import cast_fp32_to_fp32r
        flat = a.reshape(-1).view(np.uint32)
        out = np.asarray(cast_fp32_to_fp32r(flat.size, flat), dtype=np.uint32)
        return np.ascontiguousarray(out.view(np.float32).reshape(a.shape))
    except Exception:
        return a


def make_in_map(xT, wqT, wkT, wvT, woT, bq, bk, bv):
    """Pack one core's inputs into the kernel's tiled DRAM layouts."""
    D, FH, ND, NPAIR = _D, _FH, _ND, _NPAIR
    NH = FH // 64
    FHA = NH * 65
    wva = np.zeros((D, FHA), dtype=np.float32)
    bva = np.zeros((1, FHA), dtype=np.float32)
    for h in range(NH):
        wva[:, h * 65:h * 65 + 64] = np.asarray(wvT)[:, h * 64:(h + 1) * 64]
        bva[0, h * 65:h * 65 + 64] = np.asarray(bv)[h * 64:(h + 1) * 64]
        bva[0, h * 65 + 64] = 1.0
    return {
        "xT": _round_f32r(np.asarray(xT).reshape(ND, 128, -1).transpose(1, 0, 2)),
        "wq": _round_f32r(np.asarray(wqT).reshape(ND, 128, NPAIR, 128).transpose(2, 1, 0, 3)),
        "wk": _round_f32r(np.asarray(wkT).reshape(ND, 128, NPAIR, 128).transpose(2, 1, 0, 3)),
        "wv": _round_f32r(wva.reshape(ND, 128, FHA).transpose(1, 0, 2)),
        "wo": _bf16(np.asarray(woT).reshape(NPAIR, 128, ND, 128).transpose(2, 1, 0, 3)),
        "bq": _c32(np.asarray(bq).reshape(_NPAIR, 128).T),
        "bk": _c32(np.asarray(bk).reshape(_NPAIR, 128).T),
        "bv": _round_f32r(bva),
        "onesr": np.ones((1, 128), dtype=np.float32),
    }


def unpack_out(outp_tiled, S=_S):
    """[ND, 128, S] tiled partial -> [D, S]."""
    return np.asarray(outp_tiled, dtype=np.float32).reshape(_D, S)


def _shard_inputs(x, in_proj_weight, in_proj_bias, out_w):
    w = np.asarray(in_proj_weight)
    b = np.asarray(in_proj_bias)
    ow = np.asarray(out_w)
    in_maps = []
    for c in range(_NCORES):
        bi, g = divmod(c, 2)
        sl = slice(g * _FH, (g + 1) * _FH)
        in_maps.append(make_in_map(
            xT=np.asarray(x[bi]).T,
            wqT=w[0 * _D:1 * _D][sl].T,
            wkT=w[1 * _D:2 * _D][sl].T,
            wvT=w[2 * _D:3 * _D][sl].T,
            woT=ow[:, sl].T,
            bq=b[0 * _D:1 * _D][sl],
            bk=b[1 * _D:2 * _D][sl],
            bv=b[2 * _D:3 * _D][sl],
        ))
    return in_maps


LAST_RESULTS = None


def kernel(x, in_proj_weight, in_proj_bias, out_w, out_b):
    global LAST_RESULTS
    from concourse.bass_utils import run_bass_kernel_spmd
    import os

    nc = _get_nc()
    in_maps = _shard_inputs(x, in_proj_weight, in_proj_bias, out_w)
    trace = os.environ.get("BASS_TRACE", "0") not in ("", "0")
    res = run_bass_kernel_spmd(
        nc, in_maps, core_ids=list(range(_NCORES)), trace=trace
    )
    LAST_RESULTS = res
    out_b = np.asarray(out_b, dtype=np.float32)
    out = np.empty((_B, _S, _D), dtype=np.float32)
    for b in range(_B):
        part = (unpack_out(res.results[2 * b]["outp"])
                + unpack_out(res.results[2 * b + 1]["outp"]))
        out[b] = part.T + out_b
    return out


# revision 60
# speedup vs baseline: 1.0264x; 1.0027x over previous
"""Multi-head self-attention (B=4, S=2048, D=1024, H=16) on 8 NeuronCores.

Sharding: data-parallel over batch (4 groups) x tensor-parallel over heads
(2 groups of 8 heads).  Core c handles batch b=c//2, head-group g=c%2.
Each core computes its 8 heads' attention plus a partial out-projection;
the host sums the two partials per batch, transposes, adds out_b.

Per-core schedule (v4 — engine-balance rewrite, 498us -> 361us in the
TimelineSim cost model):
  - fp32r on the PE for x/q/k/v (12-bit-mantissa fp32, single-pass full
    rate); otn/wo/outp in bf16 (out-projection rel-err ~4e-3 << 2e-2)
  - phase A: xT streamed in 512-col chunks, one DMA per chunk (xT is
    host-packed [128, ND, S] so a chunk is a single descriptor run);
    v projection and pair-0 q/k interleaved per chunk, PE starts ~6.5us
  - attention runs CH=512 chunks: both heads' scores go to ONE psum
    tile -> ONE [128,1024] exp per key-tile iteration, so the PE's
    critical path crosses a single ACT semaphore per iteration;
    the AV is software-pipelined TWO iterations behind (pt ring bufs=3)
    so its exp semaphore is always already satisfied; chunk-tail AVs
    carry into the next chunk's prologue
  - q/k projections are chopped into single-matmul micro steps (own
    2-slot PSUM ring) emitted one per iteration between the scores and
    the AV; only the k slices + q j0 are projected eagerly (before the
    pair starts) — q j1..j3 run lazily inside the pair's own chunks
    0..2, which lets the ACT-bound last pair absorb its own projection
    and shortens the PE-bound phase A (wstream bufs=4 avoids a
    DMA-queue/PE deadlock cycle through the lazy readers)
  - softmax denominators ride the AV as the ones column (row 64);
    normalization per chunk: PSUM-escape copies on DVE, denominator
    broadcast via a DRAM bounce (partition-stride-0 reads), O_B's
    partition move via one SBUF-SBUF DMA, then an in-place
    reciprocal_approx_fast + multiply that are DEFERRED to the middle
    of the next chunk so their DMA wait never head-of-line-blocks the
    DVE queue
  - otn (normalized attention output) stays resident in SBUF: the out
    projection reads it directly (no DRAM round trip); out-projection
    batches for tokens 0:1024 run as fillers inside pair 3's last two
    chunks, the rest alternate ACT/DVE copies in the final phase
  - qkT is ping-ponged (2 pairs) instead of holding all 4 pairs
Weights/outputs use host-prepacked tiled layouts so every DMA is
contiguous; walrus requires Bacc.compile() for the 1-wait-per-
instruction sync legalization.
"""

import numpy as np

_B, _S, _D, _H = 4, 2048, 1024, 16
_FH = 512  # local feature dims per core (8 heads x 64)
_ND = _D // 128
_NPAIR = _FH // 128
_NCORES = 8

_CACHE = {}


def _build(S):
    import concourse.bass as bass
    import concourse.bacc as bacc
    import concourse.tile as tile
    import concourse.mybir as mybir
    from contextlib import ExitStack

    f32 = mybir.dt.float32
    f32r = mybir.dt.float32r
    bf16 = mybir.dt.bfloat16
    Exp = mybir.ActivationFunctionType.Exp
    D, FH = _D, _FH
    ND = D // 128            # contraction tiles for the projections
    NPAIR = FH // 128        # head pairs
    NKT = S // 128           # key tiles
    CH = min(512, S)         # tq chunk
    NCH = S // CH
    HW = min(512, CH)        # matmul moving free dim
    NHALF = CH // HW
    TS = min(512, S)         # projection t-slice
    NTS = S // TS
    NH = FH // 64            # local heads
    FHA = NH * 65            # v width incl. per-head ones column
    XC = min(512, S)
    NXC = S // XC

    nc = bacc.Bacc("TRN2", target_bir_lowering=False, debug=False)

    xT_d = nc.dram_tensor("xT", [128, ND, S], f32r, kind="ExternalInput")
    wq_d = nc.dram_tensor("wq", [NPAIR, 128, ND, 128], f32r, kind="ExternalInput")
    wk_d = nc.dram_tensor("wk", [NPAIR, 128, ND, 128], f32r, kind="ExternalInput")
    wv_d = nc.dram_tensor("wv", [128, ND, FHA], f32r, kind="ExternalInput")
    wo_d = nc.dram_tensor("wo", [ND, 128, NPAIR, 128], bf16, kind="ExternalInput")
    bq_d = nc.dram_tensor("bq", [128, NPAIR], f32, kind="ExternalInput")
    bk_d = nc.dram_tensor("bk", [128, NPAIR], f32, kind="ExternalInput")
    bv_d = nc.dram_tensor("bv", [1, FHA], f32r, kind="ExternalInput")
    onr_d = nc.dram_tensor("onesr", [1, 128], f32r, kind="ExternalInput")
    outp_d = nc.dram_tensor("outp", [ND, 128, S], bf16, kind="ExternalOutput")
    v_d = nc.dram_tensor("v_scr", [128, NKT, FHA - 130], f32r)

    with tile.TileContext(nc) as tc, ExitStack() as top:
        consts = top.enter_context(tc.tile_pool(name="consts", bufs=1))
        ps = top.enter_context(tc.tile_pool(name="ps", bufs=2, space="PSUM"))

        ones_row = consts.tile([1, 128], f32r)
        bv_sb = consts.tile([1, FHA], f32r)
        bqk_sb = consts.tile([128, 2 * NPAIR], f32)
        # dummy exp so the ACT table set loads during the ramp, not at the
        # first real softmax exp inside the attention window
        warm = consts.tile([1, 8], f32)
        nc.vector.memset(warm, 0.0)
        nc.scalar.activation(out=warm, in_=warm, func=Exp)

        qkT_pool = top.enter_context(tc.tile_pool(name="qk", bufs=1))
        qkT = qkT_pool.tile([128, 2, 2, S], f32r)          # [f%128, p%2, q/k, t]
        vstream = top.enter_context(tc.tile_pool(name="vstream", bufs=2))
        wstream = top.enter_context(tc.tile_pool(name="wstream", bufs=4))

        def qk_batch(p, j, which, w_sb):
            """One q-or-k projection batch: 8 accumulating matmuls + bias."""
            pps = ps.tile([128, TS], f32, tag="f")
            for d in range(ND):
                nc.tensor.matmul(
                    pps,
                    lhsT=w_sb[:, d, :],
                    rhs=xT_sb[:, d, j * TS:(j + 1) * TS],
                    start=(d == 0),
                    stop=(d == ND - 1),
                )
            nc.vector.tensor_scalar_add(
                out=qkT[:, p % 2, which, j * TS:(j + 1) * TS],
                in0=pps,
                scalar1=bqk_sb[:, which * NPAIR + p:which * NPAIR + p + 1],
            )

        def load_pair(p):
            wq_sb = wstream.tile([128, ND, 128], f32r, tag="w")
            nc.sync.dma_start(out=wq_sb, in_=wq_d[p])
            wk_sb = wstream.tile([128, ND, 128], f32r, tag="w")
            nc.sync.dma_start(out=wk_sb, in_=wk_d[p])
            if p == 0:
                v_p = v_p0
            else:
                v_p = vstream.tile([128, NKT, 130], f32r, tag="vp")
                nc.sync.dma_start(
                    out=v_p, in_=v_d[:, :, (p - 1) * 130:p * 130])
            return (wq_sb, wk_sb), v_p

        with tc.tile_pool(name="xtp", bufs=1) as xtp:
            xT_sb = xtp.tile([128, ND, S], f32r)
            v_p0 = vstream.tile([128, NKT, 130], f32r, tag="vp")

            with tc.tile_pool(name="wvp", bufs=1) as wvp, \
                    tc.tile_pool(name="vst", bufs=16) as vst:
                wv_sb = wvp.tile([128, ND, FHA], f32r)

                # ----- startup DMA priority order: xT strip 0, then wv per-d
                # (v t0's d-matmuls chase the wv arrivals), remaining strips,
                # pair-0 weights, deferred consts; later xT chunks are
                # emitted inside the phase-A loop -----
                nc.sync.dma_start(
                    out=xT_sb[:, :, 0:128], in_=xT_d[:, :, 0:128])
                for d in range(ND):
                    nc.sync.dma_start(out=wv_sb[:, d, :], in_=wv_d[:, d, :])
                    if d == 3:      # consts needed by the first bias matmul
                        nc.sync.dma_start(out=ones_row, in_=onr_d[:])
                        nc.sync.dma_start(out=bv_sb, in_=bv_d[:])
                for s in range(1, XC // 128):
                    nc.sync.dma_start(
                        out=xT_sb[:, :, s * 128:(s + 1) * 128],
                        in_=xT_d[:, :, s * 128:(s + 1) * 128])
                w0 = load_pair(0)[0]
                if NXC > 1:                       # chunk 1 right after the
                    # weights, in halves so v t4/t5 can start sooner
                    nc.sync.dma_start(
                        out=xT_sb[:, :, XC:XC + XC // 2],
                        in_=xT_d[:, :, XC:XC + XC // 2])
                    nc.sync.dma_start(
                        out=xT_sb[:, :, XC + XC // 2:2 * XC],
                        in_=xT_d[:, :, XC + XC // 2:2 * XC])
                nc.sync.dma_start(out=bqk_sb[:, 0:NPAIR], in_=bq_d[:])
                nc.sync.dma_start(out=bqk_sb[:, NPAIR:2 * NPAIR], in_=bk_d[:])

                # ----- phase A: v projection + pair-0 q/k, interleaved -----
                vsplits = [(0, min(512, FHA))]
                if FHA > 512:
                    vsplits.append((512, FHA - 512))
                for c in range(NXC):
                    if c + 2 < NXC:                   # chunk c+2 in flight
                        nc.sync.dma_start(
                            out=xT_sb[:, :, (c + 2) * XC:(c + 3) * XC],
                            in_=xT_d[:, :, (c + 2) * XC:(c + 3) * XC])
                    for t in range(4 * c, 4 * c + 4):
                        vps = ps.tile([128, FHA], f32, tag="s")
                        for c0, cw in vsplits:
                            for d in range(ND):
                                nc.tensor.matmul(
                                    vps[:, c0:c0 + cw],
                                    lhsT=xT_sb[:, d, t * 128:(t + 1) * 128],
                                    rhs=wv_sb[:, d, c0:c0 + cw],
                                    start=(d == 0),
                                    stop=False,
                                )
                            nc.tensor.matmul(
                                vps[:, c0:c0 + cw], lhsT=ones_row,
                                rhs=bv_sb[:, c0:c0 + cw], start=False, stop=True,
                            )
                        nc.vector.tensor_copy(
                            out=v_p0[:, t, :], in_=vps[:, 0:130])
                        v_st = vst.tile([128, FHA - 130], f32r, tag="vs")
                        nc.scalar.copy(out=v_st, in_=vps[:, 130:FHA])
                        nc.sync.dma_start(out=v_d[:, t, :], in_=v_st)
                    qk_batch(0, c, 1, w0[1])   # k slice c
                    if c == 0:
                        qk_batch(0, c, 0, w0[0])   # q slice 0 (j1..j3 lazy)

            # ----- attention: pairs 0..3, software-pipelined -----
            ph2 = ExitStack()
            otn_pool = ph2.enter_context(tc.tile_pool(name="otn", bufs=1))
            otn = otn_pool.tile([128, NPAIR, S], bf16)     # resident attn output
            pt_pool = ph2.enter_context(tc.tile_pool(name="pt", bufs=3))
            nrm_pool = ph2.enter_context(tc.tile_pool(name="nrm", bufs=4))
            wo_pool = ph2.enter_context(tc.tile_pool(name="wop", bufs=8))
            st_pool = ph2.enter_context(tc.tile_pool(name="st", bufs=3))
            rs_pool = ph2.enter_context(tc.tile_pool(name="rsp", bufs=2))
            stv_pool = ph2.enter_context(tc.tile_pool(name="stv", bufs=2))
            dr_pool = ph2.enter_context(
                tc.tile_pool(name="dr", bufs=2, space="DRAM"))
            def make_qk_fillers(p, w_tiles):
                """Micro-step emitters for pair p's q/k projection (k first).
                One N=512 d-matmul per step; 64 steps per pair = one per
                i-iteration.  The accumulator lives in its own 2-slot PSUM
                ring so spreading steps across iterations cannot jam the
                score-tile ring."""
                steps = []
                cell = {}

                def step(j, which, w_sb, d):
                    if d == 0:
                        cell["pps"] = ps.tile(
                            [128, TS], f32, tag="f", name="fpps")
                    nc.tensor.matmul(
                        cell["pps"],
                        lhsT=w_sb[:, d, :],
                        rhs=xT_sb[:, d, j * TS:(j + 1) * TS],
                        start=(d == 0),
                        stop=(d == ND - 1),
                    )
                    if d == ND - 1:
                        nc.vector.tensor_scalar_add(
                            out=qkT[:, p % 2, which, j * TS:(j + 1) * TS],
                            in0=cell["pps"],
                            scalar1=bqk_sb[:, which * NPAIR + p:
                                           which * NPAIR + p + 1],
                        )

                def unit(j, which):
                    w_sb = w_tiles[0] if which == 0 else w_tiles[1]
                    return [lambda j=j, w=which, ws=w_sb, d=d:
                            step(j, w, ws, d) for d in range(ND)]

                # eager part (must finish before pair p starts): all k
                # slices + q j0.  The q j1..j3 slices are only read by
                # pair p's chunks 1..3 and are hosted lazily inside pair
                # p's own chunks 0..2 (returned separately).
                eager = []
                for j in range(NTS):
                    eager += unit(j, 1)
                eager += unit(0, 0)
                lazy = []
                for j in range(1, NTS):
                    lazy.append(unit(j, 0))
                return eager, lazy

            wo_tiles = {}

            def make_outproj_steps(ets):
                """1-matmul out-projection micro steps for pair-3 slack.
                Each (et, j) unit: 4 accumulating N=512 matmuls in the free
                f-ring + a DVE copy and DMA.  Ordered all-j0-then-j1 so the
                j1 reads come after chunk 1's deferred norm multiply."""
                steps = []
                cell = {}

                def step(et, j, p):
                    if p == 0:
                        cell["ops"] = ps.tile(
                            [128, 512], f32, tag="f", name="ojp")
                    nc.tensor.matmul(
                        cell["ops"],
                        lhsT=wo_tiles[et][:, p, :],
                        rhs=otn[:, p, j * 512:(j + 1) * 512],
                        start=(p == 0),
                        stop=(p == NPAIR - 1),
                    )
                    if p == NPAIR - 1:
                        st = stv_pool.tile([128, 512], bf16, tag="sv",
                                           name="stj")
                        nc.vector.tensor_copy(out=st, in_=cell["ops"])
                        nc.sync.dma_start(
                            out=outp_d[et][:, j * 512:(j + 1) * 512], in_=st)

                for j in (0, 1):
                    for et in ets:
                        for p in range(NPAIR):
                            steps.append(
                                lambda et=et, j=j, p=p: step(et, j, p))
                return steps

            def load_wo(et):
                wo_sb = wo_pool.tile([128, NPAIR, 128], bf16, tag="wo")
                nc.sync.dma_start(out=wo_sb, in_=wo_d[et])
                wo_tiles[et] = wo_sb

            OW = 1024          # out-projection batch token width

            def outproj_batch(et, jj, copy_eng, split=False):
                ops = ps.tile([128, OW], f32, tag="s")
                for h in range(2):
                    j = 2 * jj + h
                    for p in range(NPAIR):
                        nc.tensor.matmul(
                            ops[:, h * 512:(h + 1) * 512],
                            lhsT=wo_tiles[et][:, p, :],
                            rhs=otn[:, p, j * 512:(j + 1) * 512],
                            start=(p == 0),
                            stop=(p == NPAIR - 1),
                        )
                if split:
                    if copy_eng == "v":
                        st = stv_pool.tile([128, OW], bf16, tag="sv")
                    else:
                        st = st_pool.tile([128, OW], bf16, tag="st")
                    nc.scalar.copy(out=st[:, 0:512], in_=ops[:, 0:512])
                    nc.vector.tensor_copy(
                        out=st[:, 512:1024], in_=ops[:, 512:1024])
                    nc.sync.dma_start(
                        out=outp_d[et][:, 2 * jj * 512:(2 * jj + 1) * 512],
                        in_=st[:, 0:512])
                    nc.sync.dma_start(
                        out=outp_d[et][:, (2 * jj + 1) * 512:(2 * jj + 2) * 512],
                        in_=st[:, 512:1024])
                    return
                if copy_eng == "v":
                    st = stv_pool.tile([128, OW], bf16, tag="sv")
                    nc.vector.tensor_copy(out=st, in_=ops)
                else:
                    st = st_pool.tile([128, OW], bf16, tag="st")
                    nc.scalar.copy(out=st, in_=ops)
                nc.sync.dma_start(
                    out=outp_d[et][:, 2 * jj * 512:(2 * jj + 2) * 512], in_=st)

            def attention_chunk(p, ch, v_p, carry, fillers,
                                fill_at=frozenset(range(NKT))):
                """Emit one CH-token chunk; returns the carry closure that the
                next chunk's prologue invokes (tail AV + normalization)."""
                t0 = ch * CH
                oA = ps.tile([128, CH], f32, tag="o")
                oB = ps.tile([128, CH], f32, tag="o")
                slot = p % 2
                prev = None

                def emit_scores(i):
                    # both heads' scores into one PSUM tile -> ONE exp per
                    # iteration (single semaphore on the PE's critical path)
                    s2 = ps.tile([128, 2 * CH], f32, tag="s")
                    kslc = slice(i * 128, (i + 1) * 128)
                    for half, lo in ((0, 0), (1, 64)):
                        nc.tensor.matmul(
                            s2[:, half * CH:(half + 1) * CH],
                            lhsT=qkT[lo:lo + 64, slot, 1, kslc],
                            rhs=qkT[lo:lo + 64, slot, 0, t0:t0 + CH],
                            start=True, stop=True,
                            tile_position=(lo, 0),
                        )
                    pt = pt_pool.tile([128, 2 * CH], f32r, tag="pt")
                    nc.scalar.activation(out=pt, in_=s2, func=Exp, scale=0.125)
                    return pt

                def emit_av(rec, half):
                    i, pt = rec
                    first, last = (i == 0), (i == NKT - 1)
                    ox = oA if half == 0 else oB
                    vw = slice(0, 65) if half == 0 else slice(65, 130)
                    nc.tensor.matmul(
                        ox[0:65, :], lhsT=v_p[:, i, vw],
                        rhs=pt[:, half * CH:(half + 1) * CH],
                        start=first, stop=last,
                    )

                pend = []          # AV emission lags TWO iterations so the
                for i in range(NKT):   # pt sem is always satisfied already
                    pt = emit_scores(i)
                    if i == 0 and carry is not None:
                        carry(0)
                    if i in fill_at and fillers:
                        fillers.pop(0)()
                    if len(pend) >= 2:
                        emit_av(pend[0], 0)
                    if i == 0 and carry is not None:
                        carry(1)
                    if len(pend) >= 2:
                        emit_av(pend.pop(0), 1)
                    if i == 8 and carry is not None:
                        carry(2)
                        carry = None
                    pend.append((i, pt))

                nrm = {}

                def new_carry(phase):
                    if phase == 0:
                        emit_av(pend[0], 0)
                        emit_av(pend[0], 1)
                        return
                    if phase == 1:
                        emit_av(pend[1], 0)
                        emit_av(pend[1], 1)
                        # --- normalization part 1: PSUM escape + denominator
                        # broadcast via a DRAM bounce (DMA-only tail) ---
                        aS = nrm_pool.tile([128, CH], f32, tag="n")
                        nc.vector.tensor_copy(out=aS[0:65, :], in_=oA[0:65, :])
                        bS = nrm_pool.tile([128, CH], f32, tag="n")
                        nc.vector.tensor_copy(out=bS[0:65, :], in_=oB[0:65, :])
                        dscr = dr_pool.tile([2, CH], f32, tag="d")
                        nc.sync.dma_start(out=dscr[0:1, :], in_=aS[64:65, :])
                        nc.sync.dma_start(out=dscr[1:2, :], in_=bS[64:65, :])
                        nc.sync.dma_start(out=aS[64:128, :], in_=bS[0:64, :])
                        rS = rs_pool.tile([128, CH], f32, tag="rs")
                        nc.sync.dma_start(
                            out=rS[0:64, :],
                            in_=dscr[0:1, :].to_broadcast([64, CH]))
                        nc.sync.dma_start(
                            out=rS[64:128, :],
                            in_=dscr[1:2, :].to_broadcast([64, CH]))
                        nrm.update(aS=aS, rS=rS)
                        return
                    # phase 2 (deferred to mid-next-chunk so the recip's DMA
                    # wait never head-of-line-blocks the DVE queue)
                    nc.vector.reciprocal_approx_fast(
                        out=nrm["rS"], in_=nrm["rS"])
                    nc.vector.tensor_mul(
                        out=otn[:, p, t0:t0 + CH], in0=nrm["aS"], in1=nrm["rS"])
                return new_carry

            w_cur, v_cur = w0, v_p0
            lazy0 = make_qk_fillers(0, w0)[1]
            lazy_cur = lazy0            # pair p's own q j1..j3 slices
            carry = None
            for p in range(NPAIR):
                if p + 1 < NPAIR:
                    w_nxt, v_nxt = load_pair(p + 1)
                    eager, lazy_nxt = make_qk_fillers(p + 1, w_nxt)
                else:
                    w_nxt = v_nxt = None
                    for et in range(8):
                        load_wo(et)
                    eager, lazy_nxt = [], None
                if p == NPAIR - 1:
                    osteps = make_outproj_steps(range(4))
                for ch in range(NCH):
                    if p == NPAIR - 1 and ch >= NCH - 2:
                        # out-proj micro steps over tokens 0:1024; first 16
                        # in chunk 2 (j1 reads land after the i==8 deferred
                        # norm), remaining 8 spread over chunk 3
                        fl = osteps[:16]
                        osteps = osteps[16:]
                        fa = (frozenset(range(NKT)) if len(fl) >= NKT else
                              frozenset(round(k * NKT / len(fl))
                                        for k in range(len(fl))))
                        carry = attention_chunk(
                            p, ch, v_cur, carry, fl, fill_at=fa)
                        continue
                    # lazy q j(ch+1) first (read by the NEXT chunk), then
                    # this chunk's share of the next pair's eager steps
                    fl = []
                    if lazy_cur:
                        if p == NPAIR - 1:
                            parts = {0: [0], 1: [1, 2]}.get(ch, [])
                        else:
                            parts = [ch] if ch < len(lazy_cur) else []
                        for ix in parts:
                            fl += lazy_cur[ix]
                    take = NKT - len(fl)
                    fl += eager[:take]
                    eager = eager[take:]
                    if len(fl) < NKT:    # spread sparse fillers evenly
                        fa = frozenset(
                            round(k * NKT / len(fl)) for k in range(len(fl)))
                        carry = attention_chunk(
                            p, ch, v_cur, carry, fl, fill_at=fa)
                    else:
                        carry = attention_chunk(p, ch, v_cur, carry, fl)
                w_cur, v_cur = w_nxt, v_nxt
                lazy_cur = lazy_nxt
            carry(0)
            carry(1)
            carry(2)

            # ----- out projection (remaining batches; copies alternate
            # ACT / DVE so the tail drains through two engines) -----
            alt = 0
            for et in range(4, 8):
                outproj_batch(et, 0, "s" if alt % 2 == 0 else "v")
                alt += 1
            for et in range(8):
                outproj_batch(et, 1, "s" if alt % 2 == 0 else "v",
                              split=(et >= 6))
                alt += 1
            ph2.close()

    nc.compile()
    return nc


def _get_nc(S=_S):
    if S not in _CACHE:
        _CACHE[S] = _build(S)
    return _CACHE[S]


def _c32(a):
    return np.ascontiguousarray(a, dtype=np.float32)


def _bf16(a):
    import ml_dtypes
    return np.ascontiguousarray(
        np.asarray(a, dtype=np.float32).astype(ml_dtypes.bfloat16))


def _round_f32r(a):
    """Round fp32 -> nearest fp32r (12-bit mantissa) so PE fp32r matmuls
    see properly rounded operands."""
    a = _c32(a)
    try:
        from neuron_dtypes._impl.fp32r import cast_fp32_to_fp32r
        flat = a.reshape(-1).view(np.uint32)
        out = np.asarray(cast_fp32_to_fp32r(flat.size, flat), dtype=np.uint32)
        return np.ascontiguousarray(out.view(np.float32).reshape(a.shape))
    except Exception:
        return a


def make_in_map(xT, wqT, wkT, wvT, woT, bq, bk, bv):
    """Pack one core's inputs into the kernel's tiled DRAM layouts."""
    D, FH, ND, NPAIR = _D, _FH, _ND, _NPAIR
    NH = FH // 64
    FHA = NH * 65
    wva = np.zeros((D, FHA), dtype=np.float32)
    bva = np.zeros((1, FHA), dtype=np.float32)
    for h in range(NH):
        wva[:, h * 65:h * 65 + 64] = np.asarray(wvT)[:, h * 64:(h + 1) * 64]
        bva[0, h * 65:h * 65 + 64] = np.asarray(bv)[h * 64:(h + 1) * 64]
        bva[0, h * 65 + 64] = 1.0
    return {
        "xT": _round_f32r(np.asarray(xT).reshape(ND, 128, -1).transpose(1, 0, 2)),
        "wq": _round_f32r(np.asarray(wqT).reshape(ND, 128, NPAIR, 128).transpose(2, 1, 0, 3)),
        "wk": _round_f32r(np.asarray(wkT).reshape(ND, 128, NPAIR, 128).transpose(2, 1, 0, 3)),
        "wv": _round_f32r(wva.reshape(ND, 128, FHA).transpose(1, 0, 2)),
        "wo": _bf16(np.asarray(woT).reshape(NPAIR, 128, ND, 128).transpose(2, 1, 0, 3)),
        "bq": _c32(np.asarray(bq).reshape(_NPAIR, 128).T),
        "bk": _c32(np.asarray(bk).reshape(_NPAIR, 128).T),
        "bv": _round_f32r(bva),
        "onesr": np.ones((1, 128), dtype=np.float32),
    }


def unpack_out(outp_tiled, S=_S):
    """[ND, 128, S] tiled partial -> [D, S]."""
    return np.asarray(outp_tiled, dtype=np.float32).reshape(_D, S)


def _shard_inputs(x, in_proj_weight, in_proj_bias, out_w):
    w = np.asarray(in_proj_weight)
    b = np.asarray(in_proj_bias)
    ow = np.asarray(out_w)
    in_maps = []
    for c in range(_NCORES):
        bi, g = divmod(c, 2)
        sl = slice(g * _FH, (g + 1) * _FH)
        in_maps.append(make_in_map(
            xT=np.asarray(x[bi]).T,
            wqT=w[0 * _D:1 * _D][sl].T,
            wkT=w[1 * _D:2 * _D][sl].T,
            wvT=w[2 * _D:3 * _D][sl].T,
            woT=ow[:, sl].T,
            bq=b[0 * _D:1 * _D][sl],
            bk=b[1 * _D:2 * _D][sl],
            bv=b[2 * _D:3 * _D][sl],
        ))
    return in_maps


LAST_RESULTS = None


def kernel(x, in_proj_weight, in_proj_bias, out_w, out_b):
    global LAST_RESULTS
    from concourse.bass_utils import run_bass_kernel_spmd
    import os

    nc = _get_nc()
    in_maps = _shard_inputs(x, in_proj_weight, in_proj_bias, out_w)
    trace = os.environ.get("BASS_TRACE", "0") not in ("", "0")
    res = run_bass_kernel_spmd(
        nc, in_maps, core_ids=list(range(_NCORES)), trace=trace
    )
    LAST_RESULTS = res
    out_b = np.asarray(out_b, dtype=np.float32)
    out = np.empty((_B, _S, _D), dtype=np.float32)
    for b in range(_B):
        part = (unpack_out(res.results[2 * b]["outp"])
                + unpack_out(res.results[2 * b + 1]["outp"]))
        out[b] = part.T + out_b
    return out
